# revision 37
# baseline (speedup 1.0000x reference)
"""CoLA GNN model kernel for 8 Trainium2 NeuronCores.

Math (per branch, pos/neg):
  xw   = x @ W_gcn                                   [N, 256]
  agg  = scatter_add(dst, w * xw[src])               [N, 256]
  h    = PReLU(agg + b_gcn)                          [N, 256]
  pool = l2norm(mean(h over nodes 0..6 per subgraph))
  anch = l2norm(h node 7 per subgraph)               (pos branch only)
  score_b = pool_b . (W_bil @ anch_b) + b_bil

Device mapping (per core: 1024 subgraphs = 8192 nodes per branch):
  - host precomputes x^T (bf16, feature-chunk-major) and the weighted
    block-diagonal adjacency bdt[src, dst] per 128-node block (bf16);
  - per 256-node pair: xw on PE (lhsT = x^T chunk), PSUM->SBUF copy on
    Act, agg via block-diag matmul on PE, PReLU fused on DVE,
    pool/anchor transposed out via h-stationary matmuls;
  - stages software-pipelined with a skew of one/two pairs; poolt
    group copies staggered across two pairs to keep Act under PE;
  - bilinear products/ut hoisted into the pair loop's engine slack;
    device emits the 5 raw reduction sums per subgraph (via indicator
    matmuls into one [5,512] PSUM tile per half-batch); the final
    score = rw / (||pool|| * ||anch||) + b is done on host.
"""

import numpy as np
import ml_dtypes

import concourse.mybir as mybir
import concourse.tile as tile
from concourse import bacc
from concourse.bass_utils import run_bass_kernel_spmd

F32 = mybir.dt.float32
BF16 = mybir.dt.bfloat16
AX = mybir.AluOpType

N_CORES = 8
S = 8                     # nodes per subgraph
B_TOT = 8192              # subgraphs total
BC = B_TOT // N_CORES     # subgraphs per core (1024)
NC_NODES = BC * S         # nodes per core (8192)
DIN = 512
DOUT = 256
EPB = 64                  # edges per subgraph
NBLK = NC_NODES // 128    # 64 row-blocks of 128 nodes (16 subgraphs) per branch
NPAIR = NBLK // 2         # 32 block-pairs per branch
NSLAB = 16                # x^T slabs per branch (512 nodes each)
EPS = 1e-12
SUMS = ("ssa", "ssp", "ssn", "rwp", "rwn")

_KERNEL_CACHE = {}


def _build(use_bias: bool):
    nc = bacc.Bacc(None, target_bir_lowering=False)

    # ---- I/O ----
    xt_pos = nc.dram_tensor("xt_pos", [128, 4 * NC_NODES], BF16, kind="ExternalInput")
    xt_neg = nc.dram_tensor("xt_neg", [128, 4 * NC_NODES], BF16, kind="ExternalInput")
    bdt_pos = nc.dram_tensor("bdt_pos", [128, NBLK * 128], BF16, kind="ExternalInput")
    bdt_neg = nc.dram_tensor("bdt_neg", [128, NBLK * 128], BF16, kind="ExternalInput")
    wg_in = nc.dram_tensor("wg_in", [128, 4 * DOUT], BF16, kind="ExternalInput")
    wbt_in = nc.dram_tensor("wbt_in", [128, 512], BF16, kind="ExternalInput")
    pam_in = nc.dram_tensor("pam_in", [128, 32], BF16, kind="ExternalInput")
    ind5_in = nc.dram_tensor("ind5_in", [128, 25], BF16, kind="ExternalInput")
    a_in = nc.dram_tensor("a_in", [128, 1], F32, kind="ExternalInput")
    if use_bias:
        bgcn_in = nc.dram_tensor("bgcn_in", [1, DOUT], F32, kind="ExternalInput")
    sums_out = nc.dram_tensor("sums_out", [2, 5 * 512], F32, kind="ExternalOutput")

    with tile.TileContext(nc) as tc:
        with tc.tile_pool(name="const", bufs=1) as cpool, \
             tc.tile_pool(name="persist", bufs=1) as ppool:

            wg = cpool.tile([128, 4 * DOUT], BF16)
            wbt = cpool.tile([128, 512], BF16)
            pam = cpool.tile([128, 32], BF16)
            ind5 = cpool.tile([128, 25], BF16)
            a_rep = cpool.tile([128, 1], F32)
            if use_bias:
                bg_row = cpool.tile([1, DOUT], F32)
                bg_bc = cpool.tile([128, DOUT], F32)

            bdt = {}
            poolt = {}
            for br in ("pos", "neg"):
                bdt[br] = ppool.tile([128, NBLK * 128], BF16,
                                     name=f"bdt_{br}", tag=f"bdt_{br}")
                poolt[br] = [
                    ppool.tile([128, (NBLK // 16) * 512], BF16,
                               name=f"poolt_{br}{dc}", tag=f"poolt_{br}{dc}")
                    for dc in range(2)
                ]
            # bilinear intermediates (persistent, written mid-loop)
            ut_sb = {(bg, dc): ppool.tile([128, 512], BF16,
                                          name=f"ut{bg}{dc}", tag=f"ut{bg}{dc}")
                     for bg in range(2) for dc in range(2)}
            rs = {(n, bg): ppool.tile([128, 512], BF16,
                                      name=f"rs_{n}{bg}", tag=f"rs_{n}{bg}")
                  for n in SUMS for bg in range(2)}
            btmp = [ppool.tile([128, 512], BF16, name=f"btmp{i}", tag=f"btmp{i}")
                    for i in range(2)]

            branches = ("pos", "neg")
            xt_dram = {"pos": xt_pos, "neg": xt_neg}
            bdt_dram = {"pos": bdt_pos, "neg": bdt_neg}

            def dma_bdt(br, blo, bhi):
                # gpsimd (SWDGE) queue: keeps bdt transfers out of the
                # SP FIFO so x^T slab prefetches aren't queued behind them
                nc.gpsimd.dma_start(bdt[br][:, blo * 128:bhi * 128],
                                    bdt_dram[br][:, blo * 128:bhi * 128])

            # poolt cols: 512*g + 32*m + j (pool) / +16 (anchor); b = 256*g+16*m+j
            def quarter(br, dc, bg, anchor):
                full = poolt[br][dc][:].rearrange(
                    "p (g m t) -> p g m t", g=4, m=16, t=32)
                tsl = slice(16, 32) if anchor else slice(0, 16)
                return full[:, 2 * bg:2 * bg + 2, :, tsl]

            def lin3(ap):
                return ap.rearrange("p (gg m j) -> p gg m j", gg=2, m=16)

            with tc.tile_pool(name="xp", bufs=3) as xpool, \
                 tc.tile_pool(name="blk", bufs=4) as bpool, \
                 tc.tile_pool(name="ps", bufs=2, space="PSUM") as pspool, \
                 tc.tile_pool(name="psq", bufs=2, space="PSUM") as pqpool:

                xt_tiles = {}

                def _slab(br, s):
                    t = xpool.tile([128, 4 * 512], BF16,
                                   name=f"xt_{br}{s}", tag="xt", bufs=16)
                    src = xt_dram[br][:].rearrange("p (c n) -> p c n", c=4)
                    nc.sync.dma_start(
                        t[:].rearrange("p (c n) -> p c n", c=4),
                        src[:, :, s * 512:(s + 1) * 512])
                    xt_tiles[(br, s)] = t

                # ---- bilinear hoisted pieces ----
                def bil_ut(bg, dc):
                    ps_ut = pspool.tile([128, 512], F32,
                                        name=f"ps_ut{bg}{dc}", tag="xw")
                    for ec in range(2):
                        nc.tensor.matmul(
                            ps_ut[:], wbt[:, ec * 256 + dc * 128:
                                          ec * 256 + (dc + 1) * 128],
                            quarter("pos", ec, bg, True),
                            start=(ec == 0), stop=(ec == 1))
                    nc.scalar.copy(ut_sb[(bg, dc)][:], ps_ut[:])

                def bil_prod(n, bg):
                    # rs[n,bg] = sum over dc of elementwise product
                    qa = {"ssa": lambda dc: quarter("pos", dc, bg, True),
                          "ssp": lambda dc: quarter("pos", dc, bg, False),
                          "ssn": lambda dc: quarter("neg", dc, bg, False),
                          "rwp": lambda dc: quarter("pos", dc, bg, False),
                          "rwn": lambda dc: quarter("neg", dc, bg, False)}[n]
                    for dc in range(2):
                        q = qa(dc)
                        other = (lin3(ut_sb[(bg, dc)][:])
                                 if n in ("rwp", "rwn") else q)
                        nc.vector.tensor_tensor(
                            lin3(btmp[dc][:]), q, other, AX.mult)
                    nc.vector.tensor_tensor(
                        rs[(n, bg)][:], btmp[0][:], btmp[1][:], AX.add)

                def bil_prod_g(n, bg, gg):
                    # one 256-col group-half of bil_prod (gg in {0,1} within
                    # the half-batch); lets the last half run post-loop only
                    br = "neg" if n in ("ssn", "rwn") else "pos"
                    anchor = n == "ssa"
                    cs = slice(gg * 256, (gg + 1) * 256)
                    tsl = slice(16, 32) if anchor else slice(0, 16)
                    for dc in range(2):
                        full = poolt[br][dc][:].rearrange(
                            "p (g m t) -> p g m t", g=4, m=16, t=32)
                        q = full[:, 2 * bg + gg:2 * bg + gg + 1, :, tsl]
                        if n in ("rwp", "rwn"):
                            other = lin3(ut_sb[(bg, dc)][:])[:, gg:gg + 1]
                        else:
                            other = q
                        nc.vector.tensor_tensor(
                            lin3(btmp[dc][:])[:, gg:gg + 1], q, other, AX.mult)
                    nc.vector.tensor_tensor(
                        rs[(n, bg)][:, cs], btmp[0][:, cs], btmp[1][:, cs],
                        AX.add)

                ps_sums = {}

                def bil_sums(bg, part="all"):
                    # 5 reductions into one [5,512] PSUM via indicator lhsT;
                    # "pos"/"neg" split lets the pos-side matmuls fire
                    # before the neg products land.
                    if part in ("all", "pos"):
                        ps_sums[bg] = pqpool.tile([128, 512], F32,
                                                  name=f"ps_sums{bg}",
                                                  tag="pt0")
                    ps_s = ps_sums[bg]
                    idx = {"all": (0, 1, 2, 3, 4), "pos": (0, 1, 3),
                           "neg": (2, 4)}[part]
                    for k in idx:
                        nc.tensor.matmul(ps_s[0:5, :],
                                         ind5[:, 5 * k:5 * k + 5],
                                         rs[(SUMS[k], bg)][:],
                                         start=(k == idx[0] and
                                                part in ("all", "pos")),
                                         stop=(k == idx[-1] and
                                               part in ("all", "neg")))
                    if part in ("all", "neg"):
                        ssb = bpool.tile([5, 512], F32, name=f"sums_sb{bg}",
                                         tag="sums_sb", bufs=2)
                        nc.scalar.copy(ssb[:], ps_s[0:5, :])
                        nc.sync.dma_start(
                            sums_out[bg:bg + 1, :].rearrange(
                                "r (p c) -> r p c", p=5),
                            ssb[:])

                # schedule[g] = thunks emitted just before pair g
                NG = 2 * NPAIR
                schedule = [[] for _ in range(NG + 4)]
                schedule[0].append(
                    lambda: nc.gpsimd.dma_start(pam[:], pam_in[:]))
                schedule[0].append(
                    lambda: nc.gpsimd.dma_start(wbt[:], wbt_in[:]))
                schedule[0].append(
                    lambda: nc.gpsimd.dma_start(ind5[:], ind5_in[:]))
                # bdt in 8-block chunks, spread to smooth DMA-bus load
                for c in range(2, 8):
                    schedule[max(0, 4 * c - 12)].append(
                        lambda lo=8 * c: dma_bdt("pos", lo, lo + 8))
                for c in range(8):
                    schedule[4 * c + 18].append(
                        lambda lo=8 * c: dma_bdt("neg", lo, lo + 8))
                for b in range(2):
                    for s in range(NSLAB):
                        g = 32 * b + 2 * s - 16
                        if g >= 0:
                            schedule[g].append(
                                lambda br=branches[b], s=s: _slab(br, s))
                # hoisted bilinear work (deps: pos poolt grp0/1 by g~18,
                # grp2/3 by g~36; neg grp0/1 by g~50). Positions avoid the
                # post-group-boundary pairs where Act does poolt copies.
                schedule[20].append(lambda: bil_ut(0, 0))
                schedule[21].append(lambda: bil_ut(0, 1))
                schedule[22].append(lambda: bil_prod("ssa", 0))
                schedule[23].append(lambda: bil_prod("ssp", 0))
                schedule[24].append(lambda: bil_prod("rwp", 0))
                schedule[36].append(lambda: bil_ut(1, 0))
                schedule[37].append(lambda: bil_ut(1, 1))
                schedule[38].append(lambda: bil_prod("ssa", 1))
                schedule[39].append(lambda: bil_prod("ssp", 1))
                schedule[40].append(lambda: bil_prod("rwp", 1))
                schedule[52].append(lambda: bil_prod("ssn", 0))
                schedule[53].append(lambda: bil_prod("rwn", 0))
                schedule[55].append(lambda: bil_sums(0))
                schedule[61].append(lambda: bil_prod_g("ssn", 1, 0))
                schedule[62].append(lambda: bil_prod_g("rwn", 1, 0))

                # head: minimal serial prefix + PE p-state warmup. ~400
                # tiny matmuls keep PE continuously busy through the DMA
                # head so real matmuls start fully ramped.
                warm = bpool.tile([128, 16], BF16, name="warm", tag="warm")
                nc.vector.memset(warm[:], 0.0)
                one_f = bpool.tile([128, 1], F32, name="one_f", tag="one_f")
                nc.vector.memset(one_f[:], 1.0)
                ps_warm = pqpool.tile([128, 512], F32, name="ps_warm",
                                      tag="pt0")
                for _ in range(400):
                    nc.tensor.matmul(ps_warm[0:16, 0:16], warm[:],
                                     warm[:], start=True, stop=True)
                # dummy reader so the verifier sees ps_warm consumed
                nc.vector.tensor_copy(btmp[0][0:16, 0:16],
                                      ps_warm[0:16, 0:16])
                nc.sync.dma_start(wg[:], wg_in[:])
                _slab("pos", 0)
                _slab("pos", 1)
                dma_bdt("pos", 0, 16)
                nc.sync.dma_start(a_rep[:], a_in[:])
                if use_bias:
                    nc.sync.dma_start(bg_row[:], bgcn_in[:])
                    nc.gpsimd.partition_broadcast(bg_bc[:], bg_row[:])
                for s in range(2, 8):
                    _slab("pos", s)

                state = {}
                ps_pt = {}
                pending_copy = []   # staggered poolt copies

                def stage_xw(g):
                    br = branches[g // NPAIR]
                    B0 = 2 * (g % NPAIR)
                    xt = xt_tiles[(br, B0 // 4)]
                    ps_xw = pspool.tile([128, 2 * DOUT], F32,
                                        name=f"ps_xw{g}", tag="xw")
                    for half in range(2):
                        bb = (B0 + half) % 4
                        for k in range(4):
                            nc.tensor.matmul(
                                ps_xw[:, half * DOUT:(half + 1) * DOUT],
                                xt[:, k * 512 + bb * 128:
                                   k * 512 + (bb + 1) * 128],
                                wg[:, k * DOUT:(k + 1) * DOUT],
                                start=(k == 0), stop=(k == 3))
                    xw_sb = bpool.tile([128, 2 * DOUT], BF16,
                                       name=f"xw_sb{g}", tag="xw_sb", bufs=4)
                    nc.vector.tensor_copy(xw_sb[:], ps_xw[:])
                    state[g] = {"br": br, "B0": B0, "xw_sb": xw_sb}

                def stage_agg(g):
                    st = state[g]
                    br, B0, xw_sb = st["br"], st["B0"], st["xw_sb"]
                    ps_agg = pspool.tile([128, 2 * DOUT], F32,
                                         name=f"ps_agg{g}", tag="agg")
                    for half in range(2):
                        B = B0 + half
                        nc.tensor.matmul(
                            ps_agg[:, half * DOUT:(half + 1) * DOUT],
                            bdt[br][:, B * 128:(B + 1) * 128],
                            xw_sb[:, half * DOUT:(half + 1) * DOUT],
                            start=True, stop=True)
                    h = bpool.tile([128, 2 * DOUT], BF16,
                                   name=f"h{g}", tag="h", bufs=4)
                    if use_bias:
                        t0 = bpool.tile([128, 2 * DOUT], BF16,
                                        name=f"t0_{g}", tag="t0", bufs=4)
                        nc.vector.tensor_tensor(
                            t0[:].rearrange("p (v c) -> p v c", v=2),
                            ps_agg[:].rearrange("p (v c) -> p v c", v=2),
                            bg_bc[:].unsqueeze(1).broadcast_to((128, 2, DOUT)),
                            AX.add)
                        nc.scalar.activation(
                            h[:], t0[:], mybir.ActivationFunctionType.Prelu,
                            alpha=a_rep[:, 0:1])
                    else:
                        nc.scalar.activation(
                            h[:], ps_agg[:],
                            mybir.ActivationFunctionType.Prelu,
                            alpha=a_rep[:, 0:1])
                    st["h"] = h

                def flush_pending():
                    # last group's copies: split across Act/DVE to shorten
                    # the tail dependency chain
                    while pending_copy:
                        br_, grp_, dc_, pt_ = pending_copy.pop(0)
                        dst = poolt[br_][dc_][:, grp_ * 512:(grp_ + 1) * 512]
                        if dc_ == 0:
                            nc.scalar.copy(dst, pt_[dc_][:])
                        else:
                            nc.vector.tensor_copy(dst, pt_[dc_][:])
                            del ps_pt[(br_, grp_)]

                def stage_pool(g):
                    if pending_copy:
                        br_, grp_, dc_, pt_ = pending_copy.pop(0)
                        nc.scalar.copy(
                            poolt[br_][dc_][:, grp_ * 512:(grp_ + 1) * 512],
                            pt_[dc_][:])
                        if dc_ == 1:
                            del ps_pt[(br_, grp_)]
                    st = state.pop(g)
                    br, B0, h = st["br"], st["B0"], st["h"]
                    grp = B0 // 16
                    if (br, grp) not in ps_pt:
                        ps_pt[(br, grp)] = [
                            pqpool.tile([128, 512], F32,
                                        name=f"pt{dc}_{br}{grp}", tag=f"pt{dc}")
                            for dc in range(2)]
                    pt = ps_pt[(br, grp)]
                    for half in range(2):
                        bi = (B0 + half) % 16
                        for dc in range(2):
                            nc.tensor.matmul(
                                pt[dc][:, bi * 32:(bi + 1) * 32],
                                h[:, half * DOUT + dc * 128:
                                  half * DOUT + (dc + 1) * 128],
                                pam[:], start=True, stop=True)
                    if B0 % 16 == 14:
                        pending_copy.append((br, grp, 0, pt))
                        pending_copy.append((br, grp, 1, pt))

                for g in range(NG + 2):
                    if g < len(schedule):
                        for th in schedule[g]:
                            th()
                    if g < NG:
                        stage_xw(g)
                    if 1 <= g <= NG:
                        stage_agg(g - 1)
                    if g >= 2:
                        stage_pool(g - 2)
                flush_pending()
                # ---- tail: bg1 last-group products + reductions + out ----
                bil_sums(1, "pos")
                bil_prod_g("ssn", 1, 1)
                bil_prod_g("rwn", 1, 1)
                bil_sums(1, "neg")

    nc.finalize()
    return nc


def _prep(inputs):
    """Host-side marshalling: shard + layout + dtype prep for the 8 cores."""
    bf = ml_dtypes.bfloat16

    def xt_prep(x):
        xb = np.asarray(x, np.float32).astype(bf).view(np.uint16)
        xb = xb.reshape(N_CORES, NC_NODES, 4, 128).transpose(0, 3, 2, 1)
        return np.ascontiguousarray(xb).reshape(N_CORES, 128, 4 * NC_NODES) \
            .view(bf)

    def bdt_prep(src, dst, w):
        src = np.asarray(src).astype(np.int64)
        dst = np.asarray(dst).astype(np.int64)
        w = np.asarray(w, np.float64)
        sub = src // S
        c = (src % S) * S + (dst % S)
        A = np.bincount(sub * EPB + c, weights=w,
                        minlength=B_TOT * EPB).astype(np.float32)
        A8 = A.reshape(N_CORES, NBLK, 16, S, S)      # [core, B, j, s, d]
        out = np.zeros((N_CORES, NBLK, 16, S, 16, S), np.float32)
        for j in range(16):
            out[:, :, j, :, j, :] = A8[:, :, j]
        out = out.transpose(0, 2, 3, 1, 4, 5).reshape(N_CORES, 128, NBLK * 128)
        return np.ascontiguousarray(out).astype(bf)

    xt_pos = xt_prep(inputs["pos_x"])
    xt_neg = xt_prep(inputs["neg_x"])
    bdt_pos = bdt_prep(inputs["pos_src"], inputs["pos_dst"], inputs["pos_w"])
    bdt_neg = bdt_prep(inputs["neg_src"], inputs["neg_dst"], inputs["neg_w"])

    wg = np.asarray(inputs["W_gcn"], np.float32).astype(bf)
    wg_sb = np.ascontiguousarray(
        wg.reshape(4, 128, DOUT).transpose(1, 0, 2).reshape(128, 4 * DOUT))
    wbt = np.asarray(inputs["W_bil"], np.float32).T.astype(bf)   # [e, d]
    wbt_sb = np.ascontiguousarray(
        wbt.reshape(2, 128, 2, 128).transpose(1, 0, 2, 3).reshape(128, 512))
    pam = np.zeros((128, 32), np.float32)
    for j in range(16):
        pam[S * j:S * j + 7, j] = 1.0 / 7.0
        pam[S * j + 7, 16 + j] = 1.0
    ind5 = np.zeros((5, 5), np.float32)
    np.fill_diagonal(ind5, 1.0)
    ind5 = np.tile(ind5.reshape(1, 25), (128, 1))
    a_rep = np.full((128, 1), float(np.asarray(inputs["prelu_a"])), np.float32)
    bgcn = np.asarray(inputs["b_gcn"], np.float32).reshape(1, DOUT)
    use_bias = bool(np.any(bgcn))

    consts = {
        "wg_in": wg_sb.astype(bf), "wbt_in": wbt_sb.astype(bf),
        "pam_in": pam.astype(bf), "ind5_in": ind5.astype(bf),
        "a_in": a_rep,
    }
    if use_bias:
        consts["bgcn_in"] = bgcn

    in_maps = []
    for k in range(N_CORES):
        m = dict(consts)
        m["xt_pos"] = xt_pos[k]
        m["xt_neg"] = xt_neg[k]
        m["bdt_pos"] = bdt_pos[k]
        m["bdt_neg"] = bdt_neg[k]
        in_maps.append(m)
    return in_maps, use_bias


def kernel(**inputs):
    in_maps, use_bias = _prep(inputs)
    if use_bias not in _KERNEL_CACHE:
        _KERNEL_CACHE[use_bias] = _build(use_bias)
    nc = _KERNEL_CACHE[use_bias]
    res = run_bass_kernel_spmd(nc, in_maps, core_ids=list(range(N_CORES)))
    bbil = float(np.asarray(inputs["b_bil"]).ravel()[0])
    pos_parts, neg_parts = [], []
    for r in res.results:
        s = np.asarray(r["sums_out"], np.float64).reshape(2, 5, 512)
        ssa, ssp, ssn, rwp, rwn = (s[:, i, :] for i in range(5))  # [2, 512]
        na = np.maximum(np.sqrt(ssa), EPS)
        pos = rwp / (np.maximum(np.sqrt(ssp), EPS) * na) + bbil
        neg = rwn / (np.maximum(np.sqrt(ssn), EPS) * na) + bbil
        pos_parts.append(pos.reshape(-1))
        neg_parts.append(neg.reshape(-1))
    pos = np.concatenate(pos_parts).astype(np.float32)
    neg = np.concatenate(neg_parts).astype(np.float32)
    return pos, neg
